# revision 15
# baseline (speedup 1.0000x reference)
"""Trainium2 Bass kernel for nn_Attention_19782619365760.

Sharding: 8 cores = 2 batches x 4 head-groups (3 heads each).
Per core (b = cid//4, h0 = 3*(cid%4)):
  - QKV projections for its 3 heads (f32r matmuls, contraction c in 6x128 chunks)
  - pass 1: S^T tiles [keys, queries]; exp with per-partition log-mask bias on ACT;
    P@V with ones-augmented V column giving softmax row-sums for free
  - pass 2: S tiles [queries, keys] with mask folded in via an augmented
    contraction row (ones-row in q, logmask/scale-row in k); exp; normalize by
    reciprocal row-sums (per-partition scalar on DVE); DMA the attn slice out
  - out projection partials per head, normalized by reciprocal row-sums during
    the cross-head accumulation; host adds the 4 core partials per batch + b_proj.
"""

import numpy as np

import concourse.bass as bass
import concourse.bacc as bacc
import concourse.mybir as mybir
import concourse.tile as tile
from concourse.bass_utils import run_bass_kernel_spmd

B, N, C = 2, 2048, 768
H, HD = 12, 64
HPC = 3  # heads per core
NCORES = 8
SCALE = HD ** -0.5
NEG = -30000.0
F32 = mybir.dt.float32
F32R = mybir.dt.float32r
EXP = mybir.ActivationFunctionType.Exp

NQB = N // 128   # 16 query blocks of 128
NKC = N // 128   # 16 key chunks of 128
NCH = C // 128   # 6 contraction chunks
NTQ = N // 256   # 8 x-stripes of 256 tokens


def _r(ap):
    return ap.bitcast(F32R)


def build_bass():
    nc = bacc.Bacc("TRN2", target_bir_lowering=False)
    xt = nc.dram_tensor("xt", [128, NTQ * NCH * 256], F32R, kind="ExternalInput")
    wqk = nc.dram_tensor("wqk", [128, HPC * NCH * 128], F32, kind="ExternalInput")
    wv = nc.dram_tensor("wv", [128, NCH * 256], F32, kind="ExternalInput")
    cpack = nc.dram_tensor("cpack", [128, 256], F32, kind="ExternalInput")
    lm2 = nc.dram_tensor("lm2", [1, N], F32, kind="ExternalInput")
    wp = nc.dram_tensor("wp", [64, HPC * C], F32, kind="ExternalInput")
    attn_o = nc.dram_tensor("attn_o", [HPC * N, N], F32, kind="ExternalOutput")
    outp = nc.dram_tensor("outp", [N, C], F32, kind="ExternalOutput")

    mul = mybir.AluOpType.mult
    add = mybir.AluOpType.add

    with nc.allow_low_precision(reason="f32r matmul inputs; accumulation stays fp32 in PSUM"), tile.TileContext(nc) as tc:
        with (
            tc.tile_pool(name="consts", bufs=1) as consts,
            tc.tile_pool(name="hqk", bufs=1) as hqk_pool,
            tc.tile_pool(name="ho", bufs=1) as ho_pool,
            tc.tile_pool(name="rows", bufs=1) as rows,
            tc.tile_pool(name="epool", bufs=2) as epool,
            tc.tile_pool(name="apool", bufs=2) as apool,
            tc.tile_pool(name="opool", bufs=1) as opool,
            tc.tile_pool(name="mm", bufs=2, space="PSUM") as mmps,
            tc.tile_pool(name="opv", bufs=2, space="PSUM") as opvps,
        ):
            # ---- persistent constants ----
            cp_dma = consts.tile([128, 256], F32)
            nc.sync.dma_start(cp_dma[:], cpack[:])
            cp = consts.tile([128, 256], F32)
            nc.vector.tensor_copy(cp[:], cp_dma[:])
            lm1_sb = cp[:, 0:16]            # per-partition log-mask per key chunk
            vb_bc = cp[:, 16:208]           # v_bias broadcast across partitions
            onescol = cp[:, 208:209]
            ones1 = onescol[0:1, :]
            qb_sb = cp[0:64, 209:212]       # q_bias per head
            wp_dma = consts.tile([64, HPC * C], F32)
            nc.sync.dma_start(wp_dma[:], wp[:])
            wp_sb = consts.tile([64, HPC * C], F32R)
            nc.vector.tensor_copy(wp_sb[:], wp_dma[:])
            # all-head V [n, d] with ones-augmented col: vh[p, h*1040 + kc*65 + d]
            vh = consts.tile([128, HPC * NKC * 65], F32R)
            # per-head qT/kT [65, N]: row 64 of qT = ones, of kT = logmask/scale
            qT_sb = [
                hqk_pool.tile([65, N], F32R, tag=f"qT{h}", name=f"qT{h}")
                for h in range(HPC)
            ]
            kT_sb = [
                hqk_pool.tile([65, N], F32R, tag=f"kT{h}", name=f"kT{h}")
                for h in range(HPC)
            ]

            # ================= phase A: QKV (x streamed in 8 stripes) =========
            with tc.tile_pool(name="stagea", bufs=1) as stagea:
                wdma = stagea.tile([128, HPC * NCH * 128], F32, tag="wdma", name="wdma")
                nc.sync.dma_start(wdma[:], wqk[:])
                wqk_sb = stagea.tile([128, HPC * NCH * 128], F32R)
                nc.vector.tensor_copy(wqk_sb[:], wdma[:])
                wdma2 = stagea.tile([128, NCH * 256], F32, tag="wdma2", name="wdma2")
                nc.sync.dma_start(wdma2[:], wv[:])
                wv_sb = stagea.tile([128, NCH * 256], F32R)
                nc.vector.tensor_copy(wv_sb[:], wdma2[:])
                lm2_tmp = stagea.tile([1, N], F32, tag="lm2tmp", name="lm2tmp")
                nc.sync.dma_start(lm2_tmp[:], lm2[:])
                for h in range(HPC):
                    nc.vector.tensor_scalar(
                        qT_sb[h][64:65, :], lm2_tmp[:], 0.0, 1.0,
                        op0=mybir.AluOpType.mult, op1=mybir.AluOpType.add,
                    )
                    nc.vector.tensor_copy(kT_sb[h][64:65, :], lm2_tmp[:])

                for ntq in range(NTQ):
                    xtile = stagea.tile([128, NCH * 256], F32R, tag="xtile", bufs=2)
                    nc.sync.dma_start(
                        xtile[:], xt[:, ntq * NCH * 256 : (ntq + 1) * NCH * 256]
                    )
                    # V for both 128-chunks of this stripe, all 3 heads at once
                    for half in range(2):
                        kc = 2 * ntq + half
                        ps = mmps.tile([128, 256], F32, tag="mm", name="psv")
                        for c in range(NCH):
                            nc.tensor.matmul(
                                ps[:],
                                _r(xtile[:, c * 256 + half * 128 : c * 256 + half * 128 + 128]),
                                _r(wv_sb[:, c * 256 : (c + 1) * 256]),
                                start=(c == 0),
                                stop=(c == NCH - 1),
                            )
                        for h in range(HPC):
                            base = h * NKC * 65 + kc * 65
                            nc.vector.tensor_add(
                                vh[:, base : base + 64],
                                ps[:, h * 64 : (h + 1) * 64],
                                vb_bc[:, h * 64 : (h + 1) * 64],
                            )
                            nc.vector.tensor_copy(vh[:, base + 64 : base + 65], onescol[:])
                    # qT/kT for this stripe, per head
                    for h in range(HPC):
                        ps = mmps.tile([128, 256], F32, tag="mm", name="psqk")
                        for c in range(NCH):
                            nc.tensor.matmul(
                                ps[:],
                                _r(wqk_sb[:, (h * NCH + c) * 128 : (h * NCH + c + 1) * 128]),
                                _r(xtile[:, c * 256 : (c + 1) * 256]),
                                start=(c == 0),
                                stop=(c == NCH - 1),
                            )
                        nc.vector.tensor_scalar_add(
                            qT_sb[h][0:64, ntq * 256 : (ntq + 1) * 256],
                            ps[0:64, :],
                            qb_sb[:, h : h + 1],
                        )
                        nc.vector.tensor_copy(
                            kT_sb[h][0:64, ntq * 256 : (ntq + 1) * 256],
                            ps[64:128, :],
                        )

            # ================= phase B: attention per head ====================
            o_sb = [None] * HPC    # unnormalized O'^T [65, N] per head (row 64 = sums)
            rcp_sb = [None] * HPC  # reciprocal row-sums per q-block [128, NQB]
            for h in range(HPC):
                qT, kT = qT_sb[h], kT_sb[h]

                # ---- pass 1: S^T -> exp -> P@V (row-sums via augmented V) ----
                o_ps = [
                    opvps.tile([65, 1024], F32, tag="opv", name=f"ops{h}")
                    for _ in range(2)
                ]
                for kc in range(NKC):
                    e_sb = epool.tile([128, N], F32R, tag="e")
                    for half in range(2):
                        ps = mmps.tile([128, 1024], F32, tag="mm", name="psst")
                        for quad in range(2):
                            qt = half * 2 + quad
                            nc.tensor.matmul(
                                ps[:, quad * 512 : (quad + 1) * 512],
                                _r(kT[0:64, kc * 128 : (kc + 1) * 128]),
                                _r(qT[0:64, qt * 512 : qt * 512 + 512]),
                                start=True,
                                stop=True,
                            )
                        nc.scalar.activation(
                            e_sb[:, half * 1024 : (half + 1) * 1024],
                            ps[:],
                            EXP,
                            bias=lm1_sb[:, kc : kc + 1],
                            scale=SCALE,
                        )
                    for half in range(2):
                        for quad in range(2):
                            qt = half * 2 + quad
                            nc.tensor.matmul(
                                o_ps[half][:, quad * 512 : (quad + 1) * 512],
                                _r(vh[:, h * NKC * 65 + kc * 65 : h * NKC * 65 + (kc + 1) * 65]),
                                _r(e_sb[:, qt * 512 : qt * 512 + 512]),
                                start=(kc == 0),
                                stop=(kc == NKC - 1),
                            )

                # ---- row-sums -> reciprocals in both layouts ----
                osb = ho_pool.tile([65, N], F32R, tag=f"osb{h}", name=f"osb{h}")
                for half in range(2):
                    nc.vector.tensor_copy(
                        osb[:, half * 1024 : (half + 1) * 1024], o_ps[half][:]
                    )
                o_sb[h] = osb
                rc_row = rows.tile([1, N], F32, tag="rcrow", bufs=2)
                nc.vector.reciprocal(rc_row[:], osb[64:65, :])
                # transpose rc_row into per-q-block per-partition scalars [128, 16]
                rcp_ps = mmps.tile([128, NQB], F32, tag="mm", name="psrc")
                for qblk in range(NQB):
                    nc.tensor.matmul(
                        rcp_ps[:, qblk : qblk + 1],
                        rc_row[:, qblk * 128 : (qblk + 1) * 128],
                        ones1[:],
                        start=True,
                        stop=True,
                    )
                rcp = rows.tile([128, NQB], F32, tag=f"rcp{h}", name=f"rcp{h}")
                nc.vector.tensor_copy(rcp[:], rcp_ps[:])
                rcp_sb[h] = rcp

                # ---- pass 2: S [q, k] (mask via augmented row) -> exp -> norm ----
                for qblk in range(NQB):
                    at = apool.tile([128, N], F32, tag="at")
                    for half in range(2):
                        ps = mmps.tile([128, 1024], F32, tag="mm", name="pss2")
                        for quad in range(2):
                            kt = half * 2 + quad
                            nc.tensor.matmul(
                                ps[:, quad * 512 : (quad + 1) * 512],
                                _r(qT[:, qblk * 128 : (qblk + 1) * 128]),
                                _r(kT[:, kt * 512 : kt * 512 + 512]),
                                start=True,
                                stop=True,
                            )
                        sl = at[:, half * 1024 : (half + 1) * 1024]
                        nc.scalar.activation(sl, ps[:], EXP, scale=SCALE)
                        nc.vector.tensor_scalar_mul(sl, sl, rcp[:, qblk : qblk + 1])
                    nc.sync.dma_start(
                        attn_o[h * N + qblk * 128 : h * N + (qblk + 1) * 128, :],
                        at[:],
                    )

            # ---- projection with fused per-head normalization ----
            for nb in range(NQB):
                ot = opool.tile([128, C], F32, tag="ot")
                for half in range(2):
                    osl = ot[:, half * 384 : (half + 1) * 384]
                    for h in range(HPC):
                        ps = mmps.tile([128, 384], F32, tag="mm", name="pspj")
                        nc.tensor.matmul(
                            ps[:],
                            _r(o_sb[h][0:64, nb * 128 : (nb + 1) * 128]),
                            _r(wp_sb[:, h * C + half * 384 : h * C + (half + 1) * 384]),
                            start=True,
                            stop=True,
                        )
                        if h == 0:
                            nc.vector.tensor_scalar_mul(
                                osl, ps[:], rcp_sb[h][:, nb : nb + 1]
                            )
                        else:
                            nc.vector.scalar_tensor_tensor(
                                osl, ps[:], rcp_sb[h][:, nb : nb + 1], osl,
                                op0=mul, op1=add,
                            )
                nc.sync.dma_start(outp[nb * 128 : (nb + 1) * 128, :], ot[:])

    nc.compile()
    return nc


def _core_inputs(cid, x, mask, w_qkv, q_bias, v_bias, w_proj):
    b = cid // 4
    h0 = HPC * (cid % 4)
    # xt[p, ntq*1536 + c*256 + j] = x[b, ntq*256 + j, c*128 + p]
    xr = x[b].reshape(NTQ, 256, NCH, 128)
    xt = np.ascontiguousarray(
        xr.transpose(3, 0, 2, 1).reshape(128, NTQ * NCH * 256), np.float32
    )
    wqk = np.empty((128, HPC * NCH * 128), np.float32)
    for h in range(HPC):
        hg = h0 + h
        wq = w_qkv[hg * 64 : (hg + 1) * 64, :]          # [64, 768]
        wk = w_qkv[C + hg * 64 : C + (hg + 1) * 64, :]  # [64, 768]
        for c in range(NCH):
            blk = np.concatenate(
                [wq[:, c * 128 : (c + 1) * 128].T, wk[:, c * 128 : (c + 1) * 128].T],
                axis=1,
            )
            wqk[:, (h * NCH + c) * 128 : (h * NCH + c + 1) * 128] = blk
    wv_all = w_qkv[2 * C + h0 * 64 : 2 * C + (h0 + HPC) * 64, :]  # [192, 768]
    wv = np.zeros((128, NCH * 256), np.float32)
    for c in range(NCH):
        wv[:, c * 256 : c * 256 + 192] = wv_all[:, c * 128 : (c + 1) * 128].T
    lmf = (mask[b].astype(np.float32) - 1.0) * (-NEG)  # 0 valid, -30000 masked
    cpack = np.zeros((128, 256), np.float32)
    cpack[:, 0:16] = lmf.reshape(NKC, 128).T
    cpack[:, 16:208] = np.tile(
        v_bias[h0 * 64 : (h0 + HPC) * 64].reshape(1, HPC * HD), (128, 1)
    )
    cpack[:, 208] = 1.0
    cpack[0:64, 209:212] = q_bias[h0 * 64 : (h0 + HPC) * 64].reshape(HPC, 64).T
    lm2 = np.ascontiguousarray((lmf / SCALE).reshape(1, N))
    wpv = np.ascontiguousarray(
        w_proj[:, h0 * 64 : (h0 + HPC) * 64].T.reshape(HPC, 64, C)
        .transpose(1, 0, 2)
        .reshape(64, HPC * C)
    )
    return {
        "xt": xt,
        "wqk": wqk,
        "wv": wv,
        "cpack": cpack,
        "lm2": lm2.astype(np.float32),
        "wp": wpv.astype(np.float32),
    }


def kernel(x, mask, w_qkv, q_bias, v_bias, w_proj, b_proj, _trace=False):
    x = np.asarray(x, np.float32)
    mask = np.asarray(mask)
    w_qkv = np.asarray(w_qkv, np.float32)
    q_bias = np.asarray(q_bias, np.float32)
    v_bias = np.asarray(v_bias, np.float32)
    w_proj = np.asarray(w_proj, np.float32)
    b_proj = np.asarray(b_proj, np.float32)

    nc = build_bass()
    in_maps = [
        _core_inputs(cid, x, mask, w_qkv, q_bias, v_bias, w_proj)
        for cid in range(NCORES)
    ]
    res = run_bass_kernel_spmd(nc, in_maps, core_ids=list(range(NCORES)), trace=_trace)
    results = res.results

    attn = np.empty((B, H, N, N), np.float32)
    out = np.zeros((B, N, C), np.float32)
    for cid in range(NCORES):
        b = cid // 4
        h0 = HPC * (cid % 4)
        attn[b, h0 : h0 + HPC] = results[cid]["attn_o"].reshape(HPC, N, N)
        out[b] += results[cid]["outp"]
    out += b_proj
    if _trace:
        kernel._last_result = res
    return out, attn


# revision 16
# speedup vs baseline: 1.1348x; 1.1348x over previous
"""Trainium2 Bass kernel for nn_Attention_19782619365760.

Sharding: 8 cores = 2 batches x 4 head-groups (3 heads each).
Per core (b = cid//4, h0 = 3*(cid%4)):
  - QKV projections for its 3 heads (f32r matmuls, contraction c in 6x128 chunks)
  - pass 1: S^T tiles [keys, queries]; exp with per-partition log-mask bias on ACT;
    P@V with ones-augmented V column giving softmax row-sums for free
  - pass 2: S tiles [queries, keys] with mask folded in via an augmented
    contraction row (ones-row in q, logmask/scale-row in k); exp; normalize by
    reciprocal row-sums (per-partition scalar on DVE); DMA the attn slice out
  - out projection partials per head, normalized by reciprocal row-sums during
    the cross-head accumulation; host adds the 4 core partials per batch + b_proj.
"""

import numpy as np

import concourse.bass as bass
import concourse.bacc as bacc
import concourse.mybir as mybir
import concourse.tile as tile
from concourse.bass_utils import run_bass_kernel_spmd

B, N, C = 2, 2048, 768
H, HD = 12, 64
HPC = 3  # heads per core
NCORES = 8
SCALE = HD ** -0.5
NEG = -30000.0
F32 = mybir.dt.float32
F32R = mybir.dt.float32r
F16 = mybir.dt.float16
EXP = mybir.ActivationFunctionType.Exp

NQB = N // 128   # 16 query blocks of 128
NKC = N // 128   # 16 key chunks of 128
NCH = C // 128   # 6 contraction chunks
NTQ = N // 256   # 8 x-stripes of 256 tokens


def _r(ap):
    return ap.bitcast(F32R)


def build_bass():
    nc = bacc.Bacc("TRN2", target_bir_lowering=False)
    xt = nc.dram_tensor("xt", [128, NTQ * NCH * 256], F32R, kind="ExternalInput")
    wqk = nc.dram_tensor("wqk", [128, HPC * NCH * 128], F32, kind="ExternalInput")
    wv = nc.dram_tensor("wv", [128, NCH * 256], F32, kind="ExternalInput")
    cpack = nc.dram_tensor("cpack", [128, 256], F32, kind="ExternalInput")
    lm2 = nc.dram_tensor("lm2", [1, N], F32, kind="ExternalInput")
    wp = nc.dram_tensor("wp", [64, HPC * C], F32, kind="ExternalInput")
    attn_o = nc.dram_tensor("attn_o", [HPC * N, N], F32, kind="ExternalOutput")
    outp = nc.dram_tensor("outp", [N, C], F32, kind="ExternalOutput")

    mul = mybir.AluOpType.mult
    add = mybir.AluOpType.add

    with nc.allow_low_precision(reason="f32r matmul inputs; accumulation stays fp32 in PSUM"), tile.TileContext(nc) as tc:
        with (
            tc.tile_pool(name="consts", bufs=1) as consts,
            tc.tile_pool(name="hqk", bufs=1) as hqk_pool,
            tc.tile_pool(name="ho", bufs=1) as ho_pool,
            tc.tile_pool(name="rows", bufs=1) as rows,
            tc.tile_pool(name="epool", bufs=2) as epool,
            tc.tile_pool(name="apool", bufs=2) as apool,
            tc.tile_pool(name="opool", bufs=1) as opool,
            tc.tile_pool(name="mm", bufs=2, space="PSUM") as mmps,
            tc.tile_pool(name="opv", bufs=2, space="PSUM") as opvps,
        ):
            # ---- persistent constants ----
            cp_dma = consts.tile([128, 256], F32)
            nc.sync.dma_start(cp_dma[:], cpack[:])
            cp = consts.tile([128, 256], F32)
            nc.vector.tensor_copy(cp[:], cp_dma[:])
            lm1_sb = cp[:, 0:16]            # per-partition log-mask per key chunk
            vb_bc = cp[:, 16:208]           # v_bias broadcast across partitions
            onescol = cp[:, 208:209]
            ones1 = onescol[0:1, :]
            qb_sb = cp[0:64, 209:212]       # q_bias per head
            wp_dma = consts.tile([64, HPC * C], F32)
            nc.sync.dma_start(wp_dma[:], wp[:])
            wp_sb = consts.tile([64, HPC * C], F32R)
            nc.vector.tensor_copy(wp_sb[:], wp_dma[:])
            # all-head V [n, d] with ones-augmented col: vh[p, h*1040 + kc*65 + d]
            vh = consts.tile([128, HPC * NKC * 65], F16)
            # per-head qT/kT [65, N]: row 64 of qT = ones, of kT = logmask/scale
            qT_sb = [
                hqk_pool.tile([65, N], F16, tag=f"qT{h}", name=f"qT{h}")
                for h in range(HPC)
            ]
            kT_sb = [
                hqk_pool.tile([65, N], F16, tag=f"kT{h}", name=f"kT{h}")
                for h in range(HPC)
            ]

            # ================= phase A: QKV (x streamed in 8 stripes) =========
            with tc.tile_pool(name="stagea", bufs=1) as stagea:
                wdma = stagea.tile([128, HPC * NCH * 128], F32, tag="wdma", name="wdma")
                nc.sync.dma_start(wdma[:], wqk[:])
                wqk_sb = stagea.tile([128, HPC * NCH * 128], F32R)
                nc.vector.tensor_copy(wqk_sb[:], wdma[:])
                wdma2 = stagea.tile([128, NCH * 256], F32, tag="wdma2", name="wdma2")
                nc.sync.dma_start(wdma2[:], wv[:])
                wv_sb = stagea.tile([128, NCH * 256], F32R)
                nc.vector.tensor_copy(wv_sb[:], wdma2[:])
                lm2_tmp = stagea.tile([1, N], F32, tag="lm2tmp", name="lm2tmp")
                nc.sync.dma_start(lm2_tmp[:], lm2[:])
                for h in range(HPC):
                    nc.vector.tensor_scalar(
                        qT_sb[h][64:65, :], lm2_tmp[:], 0.0, 256.0,
                        op0=mybir.AluOpType.mult, op1=mybir.AluOpType.add,
                    )
                    nc.vector.tensor_copy(kT_sb[h][64:65, :], lm2_tmp[:])

                for ntq in range(NTQ):
                    xtile = stagea.tile([128, NCH * 256], F32R, tag="xtile", bufs=2)
                    nc.sync.dma_start(
                        xtile[:], xt[:, ntq * NCH * 256 : (ntq + 1) * NCH * 256]
                    )
                    # V for both 128-chunks of this stripe, all 3 heads at once
                    for half in range(2):
                        kc = 2 * ntq + half
                        ps = mmps.tile([128, 256], F32, tag="mm", name="psv")
                        for c in range(NCH):
                            nc.tensor.matmul(
                                ps[:],
                                _r(xtile[:, c * 256 + half * 128 : c * 256 + half * 128 + 128]),
                                _r(wv_sb[:, c * 256 : (c + 1) * 256]),
                                start=(c == 0),
                                stop=(c == NCH - 1),
                            )
                        for h in range(HPC):
                            base = h * NKC * 65 + kc * 65
                            nc.vector.tensor_add(
                                vh[:, base : base + 64],
                                ps[:, h * 64 : (h + 1) * 64],
                                vb_bc[:, h * 64 : (h + 1) * 64],
                            )
                            nc.vector.tensor_copy(vh[:, base + 64 : base + 65], onescol[:])
                    # qT/kT for this stripe, per head
                    for h in range(HPC):
                        ps = mmps.tile([128, 256], F32, tag="mm", name="psqk")
                        for c in range(NCH):
                            nc.tensor.matmul(
                                ps[:],
                                _r(wqk_sb[:, (h * NCH + c) * 128 : (h * NCH + c + 1) * 128]),
                                _r(xtile[:, c * 256 : (c + 1) * 256]),
                                start=(c == 0),
                                stop=(c == NCH - 1),
                            )
                        nc.vector.tensor_scalar_add(
                            qT_sb[h][0:64, ntq * 256 : (ntq + 1) * 256],
                            ps[0:64, :],
                            qb_sb[:, h : h + 1],
                        )
                        nc.vector.tensor_copy(
                            kT_sb[h][0:64, ntq * 256 : (ntq + 1) * 256],
                            ps[64:128, :],
                        )

            # ================= phase B: attention per head ====================
            o_sb = [None] * HPC    # unnormalized O'^T [65, N] per head (row 64 = sums)
            rcp_sb = [None] * HPC  # reciprocal row-sums per q-block [128, NQB]
            for h in range(HPC):
                qT, kT = qT_sb[h], kT_sb[h]

                # ---- pass 1: S^T -> exp -> P@V (row-sums via augmented V) ----
                o_ps = [
                    opvps.tile([65, 1024], F32, tag="opv", name=f"ops{h}")
                    for _ in range(2)
                ]
                for kc in range(NKC):
                    e_sb = epool.tile([128, N], F16, tag="e")
                    for half in range(2):
                        ps = mmps.tile([128, 1024], F32, tag="mm", name="psst")
                        for quad in range(2):
                            qt = half * 2 + quad
                            nc.tensor.matmul(
                                ps[:, quad * 512 : (quad + 1) * 512],
                                kT[0:64, kc * 128 : (kc + 1) * 128],
                                qT[0:64, qt * 512 : qt * 512 + 512],
                                start=True,
                                stop=True,
                            )
                        nc.scalar.activation(
                            e_sb[:, half * 1024 : (half + 1) * 1024],
                            ps[:],
                            EXP,
                            bias=lm1_sb[:, kc : kc + 1],
                            scale=SCALE,
                        )
                    for half in range(2):
                        for quad in range(2):
                            qt = half * 2 + quad
                            nc.tensor.matmul(
                                o_ps[half][:, quad * 512 : (quad + 1) * 512],
                                vh[:, h * NKC * 65 + kc * 65 : h * NKC * 65 + (kc + 1) * 65],
                                e_sb[:, qt * 512 : qt * 512 + 512],
                                start=(kc == 0),
                                stop=(kc == NKC - 1),
                            )

                # ---- row-sums -> reciprocals in both layouts ----
                osb = ho_pool.tile([65, N], F32R, tag=f"osb{h}", name=f"osb{h}")
                for half in range(2):
                    nc.vector.tensor_copy(
                        osb[:, half * 1024 : (half + 1) * 1024], o_ps[half][:]
                    )
                o_sb[h] = osb
                rc_row = rows.tile([1, N], F32, tag="rcrow", bufs=2)
                nc.vector.tensor_copy(rc_row[:], osb[64:65, :])
                # transpose rc_row into per-q-block per-partition scalars [128, 16]
                rcp_ps = mmps.tile([128, NQB], F32, tag="mm", name="psrc")
                for qblk in range(NQB):
                    nc.tensor.matmul(
                        rcp_ps[:, qblk : qblk + 1],
                        rc_row[:, qblk * 128 : (qblk + 1) * 128],
                        ones1[:],
                        start=True,
                        stop=True,
                    )
                rs_sb = rows.tile([128, NQB], F32, tag="rssb", bufs=2, name="rssb")
                nc.vector.tensor_copy(rs_sb[:], rcp_ps[:])
                rcp = rows.tile([128, NQB], F32, tag=f"rcp{h}", name=f"rcp{h}")
                nc.vector.reciprocal(rcp[:], rs_sb[:])
                rcp_sb[h] = rcp

                # ---- pass 2: S [q, k] (mask via augmented row) -> exp -> norm ----
                for qblk in range(NQB):
                    at = apool.tile([128, N], F32, tag="at")
                    for half in range(2):
                        ps = mmps.tile([128, 1024], F32, tag="mm", name="pss2")
                        for quad in range(2):
                            kt = half * 2 + quad
                            nc.tensor.matmul(
                                ps[:, quad * 512 : (quad + 1) * 512],
                                qT[:, qblk * 128 : (qblk + 1) * 128],
                                kT[:, kt * 512 : kt * 512 + 512],
                                start=True,
                                stop=True,
                            )
                        sl = at[:, half * 1024 : (half + 1) * 1024]
                        nc.scalar.activation(sl, ps[:], EXP, scale=SCALE)
                        nc.vector.tensor_scalar_mul(sl, sl, rcp[:, qblk : qblk + 1])
                    nc.sync.dma_start(
                        attn_o[h * N + qblk * 128 : h * N + (qblk + 1) * 128, :],
                        at[:],
                    )

            # ---- projection with fused per-head normalization ----
            for nb in range(NQB):
                ot = opool.tile([128, C], F32, tag="ot")
                for half in range(2):
                    osl = ot[:, half * 384 : (half + 1) * 384]
                    for h in range(HPC):
                        ps = mmps.tile([128, 384], F32, tag="mm", name="pspj")
                        nc.tensor.matmul(
                            ps[:],
                            _r(o_sb[h][0:64, nb * 128 : (nb + 1) * 128]),
                            _r(wp_sb[:, h * C + half * 384 : h * C + (half + 1) * 384]),
                            start=True,
                            stop=True,
                        )
                        if h == 0:
                            nc.vector.tensor_scalar_mul(
                                osl, ps[:], rcp_sb[h][:, nb : nb + 1]
                            )
                        else:
                            nc.vector.scalar_tensor_tensor(
                                osl, ps[:], rcp_sb[h][:, nb : nb + 1], osl,
                                op0=mul, op1=add,
                            )
                nc.sync.dma_start(outp[nb * 128 : (nb + 1) * 128, :], ot[:])

    nc.compile()
    return nc


def _core_inputs(cid, x, mask, w_qkv, q_bias, v_bias, w_proj):
    b = cid // 4
    h0 = HPC * (cid % 4)
    # xt[p, ntq*1536 + c*256 + j] = x[b, ntq*256 + j, c*128 + p]
    xr = x[b].reshape(NTQ, 256, NCH, 128)
    xt = np.ascontiguousarray(
        xr.transpose(3, 0, 2, 1).reshape(128, NTQ * NCH * 256), np.float32
    )
    wqk = np.empty((128, HPC * NCH * 128), np.float32)
    for h in range(HPC):
        hg = h0 + h
        wq = w_qkv[hg * 64 : (hg + 1) * 64, :]          # [64, 768]
        wk = w_qkv[C + hg * 64 : C + (hg + 1) * 64, :]  # [64, 768]
        for c in range(NCH):
            blk = np.concatenate(
                [wq[:, c * 128 : (c + 1) * 128].T, wk[:, c * 128 : (c + 1) * 128].T],
                axis=1,
            )
            wqk[:, (h * NCH + c) * 128 : (h * NCH + c + 1) * 128] = blk
    wv_all = w_qkv[2 * C + h0 * 64 : 2 * C + (h0 + HPC) * 64, :]  # [192, 768]
    wv = np.zeros((128, NCH * 256), np.float32)
    for c in range(NCH):
        wv[:, c * 256 : c * 256 + 192] = wv_all[:, c * 128 : (c + 1) * 128].T
    lmf = (mask[b].astype(np.float32) - 1.0) * (-NEG)  # 0 valid, -30000 masked
    cpack = np.zeros((128, 256), np.float32)
    cpack[:, 0:16] = lmf.reshape(NKC, 128).T
    cpack[:, 16:208] = np.tile(
        v_bias[h0 * 64 : (h0 + HPC) * 64].reshape(1, HPC * HD), (128, 1)
    )
    cpack[:, 208] = 1.0
    cpack[0:64, 209:212] = q_bias[h0 * 64 : (h0 + HPC) * 64].reshape(HPC, 64).T
    lm2 = np.ascontiguousarray((lmf / SCALE / 256.0).reshape(1, N))
    wpv = np.ascontiguousarray(
        w_proj[:, h0 * 64 : (h0 + HPC) * 64].T.reshape(HPC, 64, C)
        .transpose(1, 0, 2)
        .reshape(64, HPC * C)
    )
    return {
        "xt": xt,
        "wqk": wqk,
        "wv": wv,
        "cpack": cpack,
        "lm2": lm2.astype(np.float32),
        "wp": wpv.astype(np.float32),
    }


def kernel(x, mask, w_qkv, q_bias, v_bias, w_proj, b_proj, _trace=False):
    x = np.asarray(x, np.float32)
    mask = np.asarray(mask)
    w_qkv = np.asarray(w_qkv, np.float32)
    q_bias = np.asarray(q_bias, np.float32)
    v_bias = np.asarray(v_bias, np.float32)
    w_proj = np.asarray(w_proj, np.float32)
    b_proj = np.asarray(b_proj, np.float32)

    nc = build_bass()
    in_maps = [
        _core_inputs(cid, x, mask, w_qkv, q_bias, v_bias, w_proj)
        for cid in range(NCORES)
    ]
    res = run_bass_kernel_spmd(nc, in_maps, core_ids=list(range(NCORES)), trace=_trace)
    results = res.results

    attn = np.empty((B, H, N, N), np.float32)
    out = np.zeros((B, N, C), np.float32)
    for cid in range(NCORES):
        b = cid // 4
        h0 = HPC * (cid % 4)
        attn[b, h0 : h0 + HPC] = results[cid]["attn_o"].reshape(HPC, N, N)
        out[b] += results[cid]["outp"]
    out += b_proj
    if _trace:
        kernel._last_result = res
    return out, attn


# revision 18
# speedup vs baseline: 1.2728x; 1.1217x over previous
"""Trainium2 Bass kernel for nn_Attention_19782619365760.

Sharding: 8 cores = 2 batches x 4 head-groups (3 heads each).
Per core (b = cid//4, h0 = 3*(cid%4)):
  - QKV projections for its 3 heads (f32r matmuls, contraction c in 6x128 chunks)
  - pass 1: S^T tiles [keys, queries]; exp with per-partition log-mask bias on ACT;
    P@V with ones-augmented V column giving softmax row-sums for free
  - pass 2: S tiles [queries, keys] with mask folded in via an augmented
    contraction row (ones-row in q, logmask/scale-row in k); exp; normalize by
    reciprocal row-sums (per-partition scalar on DVE); DMA the attn slice out
  - out projection partials per head, normalized by reciprocal row-sums during
    the cross-head accumulation; host adds the 4 core partials per batch + b_proj.
"""

import numpy as np

import concourse.bass as bass
import concourse.bacc as bacc
import concourse.mybir as mybir
import concourse.tile as tile
from concourse.bass_utils import run_bass_kernel_spmd

B, N, C = 2, 2048, 768
H, HD = 12, 64
HPC = 3  # heads per core
NCORES = 8
SCALE = HD ** -0.5
NEG = -30000.0
F32 = mybir.dt.float32
F32R = mybir.dt.float32r
F16 = mybir.dt.float16
EXP = mybir.ActivationFunctionType.Exp

NQB = N // 128   # 16 query blocks of 128
NKC = N // 128   # 16 key chunks of 128
NCH = C // 128   # 6 contraction chunks
NTQ = N // 256   # 8 x-stripes of 256 tokens


def _r(ap):
    return ap.bitcast(F32R)


def build_bass():
    nc = bacc.Bacc("TRN2", target_bir_lowering=False)
    xt = nc.dram_tensor("xt", [128, NTQ * NCH * 256], F16, kind="ExternalInput")
    wqk = nc.dram_tensor("wqk", [128, HPC * NCH * 128], F16, kind="ExternalInput")
    wv = nc.dram_tensor("wv", [128, NCH * 256], F16, kind="ExternalInput")
    cpack = nc.dram_tensor("cpack", [128, 256], F32, kind="ExternalInput")
    lm2 = nc.dram_tensor("lm2", [1, N], F32, kind="ExternalInput")
    wp = nc.dram_tensor("wp", [64, HPC * C], F32, kind="ExternalInput")
    attn_o = nc.dram_tensor("attn_o", [HPC * N, N], F32, kind="ExternalOutput")
    outp = nc.dram_tensor("outp", [N, C], F32, kind="ExternalOutput")

    mul = mybir.AluOpType.mult
    add = mybir.AluOpType.add

    with nc.allow_low_precision(reason="f32r matmul inputs; accumulation stays fp32 in PSUM"), tile.TileContext(nc) as tc:
        with (
            tc.tile_pool(name="consts", bufs=1) as consts,
            tc.tile_pool(name="hqk", bufs=1) as hqk_pool,
            tc.tile_pool(name="ho", bufs=1) as ho_pool,
            tc.tile_pool(name="rows", bufs=1) as rows,
            tc.tile_pool(name="epool", bufs=4) as epool,
            tc.tile_pool(name="apool", bufs=3) as apool,
            tc.tile_pool(name="opool", bufs=1) as opool,
            tc.tile_pool(name="mm", bufs=2, space="PSUM") as mmps,
            tc.tile_pool(name="opv", bufs=2, space="PSUM") as opvps,
        ):
            # ---- persistent constants ----
            cp_dma = consts.tile([128, 256], F32)
            nc.sync.dma_start(cp_dma[:], cpack[:])
            cp = consts.tile([128, 256], F32)
            nc.vector.tensor_copy(cp[:], cp_dma[:])
            lm1_sb = cp[:, 0:16]            # per-partition log-mask per key chunk
            vb_bc = cp[:, 16:208]           # v_bias broadcast across partitions
            onescol = cp[:, 208:209]
            ones1 = onescol[0:1, :]
            qb_sb = cp[0:64, 209:212]       # q_bias per head
            wp_dma = consts.tile([64, HPC * C], F32)
            nc.sync.dma_start(wp_dma[:], wp[:])
            wp_sb = consts.tile([64, HPC * C], F32R)
            nc.vector.tensor_copy(wp_sb[:], wp_dma[:])
            # all-head V [n, d] with ones-augmented col: vh[p, h*1040 + kc*65 + d]
            vh = consts.tile([128, HPC * NKC * 65], F16)
            # per-head qT/kT [65, N]: row 64 of qT = ones, of kT = logmask/scale
            qT_sb = [
                hqk_pool.tile([65, N], F16, tag=f"qT{h}", name=f"qT{h}")
                for h in range(HPC)
            ]
            kT_sb = [
                hqk_pool.tile([65, N], F16, tag=f"kT{h}", name=f"kT{h}")
                for h in range(HPC)
            ]

            # ================= phase A: QKV (x streamed in 8 stripes) =========
            with tc.tile_pool(name="stagea", bufs=1) as stagea:
                wqk_sb = stagea.tile([128, HPC * NCH * 128], F16)
                nc.sync.dma_start(wqk_sb[:], wqk[:])
                wv_sb = stagea.tile([128, NCH * 256], F16)
                nc.sync.dma_start(wv_sb[:], wv[:])
                lm2_tmp = stagea.tile([1, N], F32, tag="lm2tmp", name="lm2tmp")
                nc.sync.dma_start(lm2_tmp[:], lm2[:])
                for h in range(HPC):
                    nc.vector.tensor_scalar(
                        qT_sb[h][64:65, :], lm2_tmp[:], 0.0, 256.0,
                        op0=mybir.AluOpType.mult, op1=mybir.AluOpType.add,
                    )
                    nc.vector.tensor_copy(kT_sb[h][64:65, :], lm2_tmp[:])

                for ntq in range(NTQ):
                    xtile = stagea.tile([128, NCH * 256], F16, tag="xtile", bufs=2)
                    nc.sync.dma_start(
                        xtile[:], xt[:, ntq * NCH * 256 : (ntq + 1) * NCH * 256]
                    )
                    # V for both 128-chunks of this stripe, all 3 heads at once
                    for half in range(2):
                        kc = 2 * ntq + half
                        ps = mmps.tile([128, 256], F32, tag="mm", name="psv")
                        for c in range(NCH):
                            nc.tensor.matmul(
                                ps[:],
                                xtile[:, c * 256 + half * 128 : c * 256 + half * 128 + 128],
                                wv_sb[:, c * 256 : (c + 1) * 256],
                                start=(c == 0),
                                stop=(c == NCH - 1),
                            )
                        for h in range(HPC):
                            base = h * NKC * 65 + kc * 65
                            nc.vector.tensor_add(
                                vh[:, base : base + 64],
                                ps[:, h * 64 : (h + 1) * 64],
                                vb_bc[:, h * 64 : (h + 1) * 64],
                            )
                            nc.vector.tensor_copy(vh[:, base + 64 : base + 65], onescol[:])
                    # qT/kT for this stripe, per head
                    for h in range(HPC):
                        ps = mmps.tile([128, 256], F32, tag="mm", name="psqk")
                        for c in range(NCH):
                            nc.tensor.matmul(
                                ps[:],
                                wqk_sb[:, (h * NCH + c) * 128 : (h * NCH + c + 1) * 128],
                                xtile[:, c * 256 : (c + 1) * 256],
                                start=(c == 0),
                                stop=(c == NCH - 1),
                            )
                        nc.vector.tensor_scalar_add(
                            qT_sb[h][0:64, ntq * 256 : (ntq + 1) * 256],
                            ps[0:64, :],
                            qb_sb[:, h : h + 1],
                        )
                        nc.vector.tensor_copy(
                            kT_sb[h][0:64, ntq * 256 : (ntq + 1) * 256],
                            ps[64:128, :],
                        )

            # ================= phase B: attention per head ====================
            o_sb = [None] * HPC    # unnormalized O'^T [65, N] per head (row 64 = sums)
            rcp_sb = [None] * HPC  # reciprocal row-sums per q-block [128, NQB]
            for h in range(HPC):
                qT, kT = qT_sb[h], kT_sb[h]

                # ---- pass 1: S^T -> exp -> P@V (row-sums via augmented V) ----
                o_ps = [
                    opvps.tile([65, 1024], F32, tag="opv", name=f"ops{h}")
                    for _ in range(2)
                ]
                for kc in range(NKC):
                    e_sb = epool.tile([128, N], F16, tag="e")
                    for half in range(2):
                        ps = mmps.tile([128, 1024], F32, tag="mm", name="psst")
                        for quad in range(2):
                            qt = half * 2 + quad
                            nc.tensor.matmul(
                                ps[:, quad * 512 : (quad + 1) * 512],
                                kT[0:64, kc * 128 : (kc + 1) * 128],
                                qT[0:64, qt * 512 : qt * 512 + 512],
                                start=True,
                                stop=True,
                            )
                        nc.scalar.activation(
                            e_sb[:, half * 1024 : (half + 1) * 1024],
                            ps[:],
                            EXP,
                            bias=lm1_sb[:, kc : kc + 1],
                            scale=SCALE,
                        )
                    for half in range(2):
                        for quad in range(2):
                            qt = half * 2 + quad
                            nc.tensor.matmul(
                                o_ps[half][:, quad * 512 : (quad + 1) * 512],
                                vh[:, h * NKC * 65 + kc * 65 : h * NKC * 65 + (kc + 1) * 65],
                                e_sb[:, qt * 512 : qt * 512 + 512],
                                start=(kc == 0),
                                stop=(kc == NKC - 1),
                            )

                # ---- row-sums -> reciprocals in both layouts ----
                osb = ho_pool.tile([65, N], F32R, tag=f"osb{h}", name=f"osb{h}")
                for half in range(2):
                    nc.vector.tensor_copy(
                        osb[:, half * 1024 : (half + 1) * 1024], o_ps[half][:]
                    )
                o_sb[h] = osb
                rc_row = rows.tile([1, N], F32, tag="rcrow", bufs=2)
                for half in range(2):
                    nc.vector.tensor_copy(
                        rc_row[0:1, half * 1024 : (half + 1) * 1024],
                        o_ps[half][64:65, :],
                    )
                # transpose rc_row into per-q-block per-partition scalars [128, 16]
                rcp_ps = mmps.tile([128, NQB], F32, tag="mm", name="psrc")
                for qblk in range(NQB):
                    nc.tensor.matmul(
                        rcp_ps[:, qblk : qblk + 1],
                        rc_row[:, qblk * 128 : (qblk + 1) * 128],
                        ones1[:],
                        start=True,
                        stop=True,
                    )
                rs_sb = rows.tile([128, NQB], F32, tag="rssb", bufs=2, name="rssb")
                nc.vector.tensor_copy(rs_sb[:], rcp_ps[:])
                rcp = rows.tile([128, NQB], F32, tag=f"rcp{h}", name=f"rcp{h}")
                nc.vector.reciprocal(rcp[:], rs_sb[:])
                rcp_sb[h] = rcp

                # ---- pass 2: S [q, k] (mask via augmented row) -> exp -> norm ----
                for qblk in range(NQB):
                    at = apool.tile([128, N], F32, tag="at")
                    for half in range(2):
                        ps = mmps.tile([128, 1024], F32, tag="mm", name="pss2")
                        for quad in range(2):
                            kt = half * 2 + quad
                            nc.tensor.matmul(
                                ps[:, quad * 512 : (quad + 1) * 512],
                                qT[:, qblk * 128 : (qblk + 1) * 128],
                                kT[:, kt * 512 : kt * 512 + 512],
                                start=True,
                                stop=True,
                            )
                        sl = at[:, half * 1024 : (half + 1) * 1024]
                        nc.scalar.activation(sl, ps[:], EXP, scale=SCALE)
                        nc.vector.tensor_scalar_mul(sl, sl, rcp[:, qblk : qblk + 1])
                    nc.sync.dma_start(
                        attn_o[h * N + qblk * 128 : h * N + (qblk + 1) * 128, :],
                        at[:],
                    )

            # ---- projection with fused per-head normalization ----
            for nb in range(NQB):
                ot = opool.tile([128, C], F32, tag="ot")
                for half in range(2):
                    osl = ot[:, half * 384 : (half + 1) * 384]
                    for h in range(HPC):
                        ps = mmps.tile([128, 384], F32, tag="mm", name="pspj")
                        nc.tensor.matmul(
                            ps[:],
                            _r(o_sb[h][0:64, nb * 128 : (nb + 1) * 128]),
                            _r(wp_sb[:, h * C + half * 384 : h * C + (half + 1) * 384]),
                            start=True,
                            stop=True,
                        )
                        if h == 0:
                            nc.vector.tensor_scalar_mul(
                                osl, ps[:], rcp_sb[h][:, nb : nb + 1]
                            )
                        else:
                            nc.vector.scalar_tensor_tensor(
                                osl, ps[:], rcp_sb[h][:, nb : nb + 1], osl,
                                op0=mul, op1=add,
                            )
                nc.sync.dma_start(outp[nb * 128 : (nb + 1) * 128, :], ot[:])

    nc.compile()
    return nc


def _core_inputs(cid, x, mask, w_qkv, q_bias, v_bias, w_proj):
    b = cid // 4
    h0 = HPC * (cid % 4)
    # xt[p, ntq*1536 + c*256 + j] = x[b, ntq*256 + j, c*128 + p]
    xr = x[b].reshape(NTQ, 256, NCH, 128)
    xt = np.ascontiguousarray(
        xr.transpose(3, 0, 2, 1).reshape(128, NTQ * NCH * 256)
    ).astype(np.float16)
    wqk = np.empty((128, HPC * NCH * 128), np.float16)
    for h in range(HPC):
        hg = h0 + h
        wq = w_qkv[hg * 64 : (hg + 1) * 64, :]          # [64, 768]
        wk = w_qkv[C + hg * 64 : C + (hg + 1) * 64, :]  # [64, 768]
        for c in range(NCH):
            blk = np.concatenate(
                [wq[:, c * 128 : (c + 1) * 128].T, wk[:, c * 128 : (c + 1) * 128].T],
                axis=1,
            )
            wqk[:, (h * NCH + c) * 128 : (h * NCH + c + 1) * 128] = blk
    wv_all = w_qkv[2 * C + h0 * 64 : 2 * C + (h0 + HPC) * 64, :]  # [192, 768]
    wv = np.zeros((128, NCH * 256), np.float16)
    for c in range(NCH):
        wv[:, c * 256 : c * 256 + 192] = wv_all[:, c * 128 : (c + 1) * 128].T
    lmf = (mask[b].astype(np.float32) - 1.0) * (-NEG)  # 0 valid, -30000 masked
    cpack = np.zeros((128, 256), np.float32)
    cpack[:, 0:16] = lmf.reshape(NKC, 128).T
    cpack[:, 16:208] = np.tile(
        v_bias[h0 * 64 : (h0 + HPC) * 64].reshape(1, HPC * HD), (128, 1)
    )
    cpack[:, 208] = 1.0
    cpack[0:64, 209:212] = q_bias[h0 * 64 : (h0 + HPC) * 64].reshape(HPC, 64).T
    lm2 = np.ascontiguousarray((lmf / SCALE / 256.0).reshape(1, N))
    wpv = np.ascontiguousarray(
        w_proj[:, h0 * 64 : (h0 + HPC) * 64].T.reshape(HPC, 64, C)
        .transpose(1, 0, 2)
        .reshape(64, HPC * C)
    )
    return {
        "xt": xt,
        "wqk": wqk,
        "wv": wv,
        "cpack": cpack,
        "lm2": lm2.astype(np.float32),
        "wp": wpv.astype(np.float32),
    }


def kernel(x, mask, w_qkv, q_bias, v_bias, w_proj, b_proj, _trace=False):
    x = np.asarray(x, np.float32)
    mask = np.asarray(mask)
    w_qkv = np.asarray(w_qkv, np.float32)
    q_bias = np.asarray(q_bias, np.float32)
    v_bias = np.asarray(v_bias, np.float32)
    w_proj = np.asarray(w_proj, np.float32)
    b_proj = np.asarray(b_proj, np.float32)

    nc = build_bass()
    in_maps = [
        _core_inputs(cid, x, mask, w_qkv, q_bias, v_bias, w_proj)
        for cid in range(NCORES)
    ]
    res = run_bass_kernel_spmd(nc, in_maps, core_ids=list(range(NCORES)), trace=_trace)
    results = res.results

    attn = np.empty((B, H, N, N), np.float32)
    out = np.zeros((B, N, C), np.float32)
    for cid in range(NCORES):
        b = cid // 4
        h0 = HPC * (cid % 4)
        attn[b, h0 : h0 + HPC] = results[cid]["attn_o"].reshape(HPC, N, N)
        out[b] += results[cid]["outp"]
    out += b_proj
    if _trace:
        kernel._last_result = res
    return out, attn
